# revision 18
# baseline (speedup 1.0000x reference)
"""Trainium2 Bass kernel for BertForMultilabelNER head (gather + fused per-attribute classifier).

Reference computation:
    pooled = take_along_axis(sequence_output, word_idxs, axis=1)   # [B, W, D]
    logits = einsum("bwd,acd->abwc", pooled, cls_w) + cls_b        # [A, B, W, 3]

Strategy (8 NeuronCores, data-parallel over batch, 8 examples/core):
  * Host splits sequence_output rows into hi/lo bf16 planes (hi = bf16(x),
    lo = bf16(x - hi)) concatenated per row -> [B*S, 2D] bf16. Same bytes as
    f32, but enables dma_gather(transpose=True) (2-byte dtypes only), which
    gathers each indexed row directly into the [d%128 partitions, d//128, row]
    layout a matmul k-tile wants -- no on-chip transpose needed.
  * The first dma_gather cannot start before the ~17us GPSIMD library-overlay
    load, so the first NPRE*128 rows are fetched with indirect_dma_start
    (mainline SWDGE, resident firmware, starts ~10us) in natural [row, d]
    layout and transposed on the then-idle PE (via identity matmul) + DVE
    during the dead window.
  * 18 accumulating bf16 matmuls per row-chunk: hi@w_hi + hi@w_lo + lo@w_hi
    (the dropped lo@w_lo term is ~2^-18 relative) ~= full f32 precision,
    at 1 cycle/row instead of f32's 4 cycles/row on the PE.
  * Remaining rows stream via dma_gather chunks spread over the 4 SWDGE
    queues (parallel descriptor generation on distinct Q7 pairs; queue choice
    must match Tile's DMASW semaphore-lane round-robin). DVE adds the bias
    during the PSUM->SBUF copy; outputs stream back per chunk.
  * Device output is [96, 2048] per core; host reassembles [A, B, W, 3].
"""

import numpy as np
import ml_dtypes

import concourse.bass as bass
import concourse.mybir as mybir
import concourse.tile as tile
from concourse import bacc, bass_utils

B, S, W, D, A = 64, 512, 256, 768, 32
NCORES = 8
BLOC = B // NCORES            # examples per core
ROWS = BLOC * W               # gathered rows per core (2048)
NPRE = 3                      # 128-row groups pre-gathered via indirect DMA
PRE = NPRE * 128
# dma_gather chunk sizes for the remaining rows. Descriptor generation for
# all of these is gated at ~17us; big chunks go first on distinct queues
# (parallel generation, efficient single-packet transfers), small ones on
# queue 0. Queue choice per chunk must equal the queue Tile's DMASW lane
# round-robin locks that op's lane to (NPRE indirect DMAs occupy lanes
# 0..NPRE-1 on queue 0 first).
SIZES = [384, 384, 384, 128, 256, 128]
QUEUES = [1, 2, 3, 0, 1, 0]
assert sum(SIZES) == ROWS - PRE
KT = D // 128                 # k-tiles per bf16 plane (6)
NT = 3 * KT                   # matmul k-tiles: hi*whi, hi*wlo, lo*whi (18)
M = 3 * A                     # output features (96)

_cache = {}


def _build():
    nc = bacc.Bacc("TRN2", target_bir_lowering=False, debug=False,
                   num_swdge_queues=4)
    seq = nc.dram_tensor("seqcat", [BLOC * S, 2 * D], mybir.dt.bfloat16,
                         kind="ExternalInput")
    idx = nc.dram_tensor("idx", [128, ROWS // 16], mybir.dt.int16,
                         kind="ExternalInput")
    pofs = nc.dram_tensor("pofs", [128, NPRE], mybir.dt.int32,
                          kind="ExternalInput")
    wts = nc.dram_tensor("wts", [128, NT * M], mybir.dt.bfloat16,
                         kind="ExternalInput")
    bias = nc.dram_tensor("bias", [M, 1], mybir.dt.float32,
                          kind="ExternalInput")
    ident = nc.dram_tensor("ident", [128, 128], mybir.dt.bfloat16,
                           kind="ExternalInput")
    out = nc.dram_tensor("out", [M, ROWS], mybir.dt.float32,
                         kind="ExternalOutput")

    with tile.TileContext(nc) as tc:
        with (
            tc.tile_pool(name="const", bufs=1) as const_pool,
            tc.tile_pool(name="pre", bufs=1) as pre_pool,
            tc.tile_pool(name="gather", bufs=len(SIZES)) as gather_pool,
            tc.tile_pool(name="psum", bufs=5, space="PSUM") as psum_pool,
            tc.tile_pool(name="tpp", bufs=2, space="PSUM") as tp_pool,
            tc.tile_pool(name="outp", bufs=7) as out_pool,
        ):
            pofs_sb = const_pool.tile([128, NPRE], mybir.dt.int32)
            idx_sb = const_pool.tile([128, ROWS // 16], mybir.dt.int16)
            w_sb = const_pool.tile([128, NT * M], mybir.dt.bfloat16)
            b_sb = const_pool.tile([M, 1], mybir.dt.float32)
            id_sb = const_pool.tile([128, 128], mybir.dt.bfloat16)
            nc.sync.dma_start(pofs_sb[:], pofs.ap())
            nc.sync.dma_start(idx_sb[:], idx.ap())
            nc.sync.dma_start(w_sb[:], wts.ap())
            nc.sync.dma_start(b_sb[:], bias.ap())
            nc.sync.dma_start(id_sb[:], ident.ap())
            w3 = w_sb[:].rearrange("p (t n) -> p t n", t=NT)

            # --- pre-gathered head: indirect DMA (not gated by the library
            # load) + PE transpose into the k-tile layout.
            pre_t = pre_pool.tile([128, 2 * KT * PRE], mybir.dt.bfloat16)
            pre3 = pre_t[:].rearrange("p (j n) -> p j n", j=2 * KT)
            for g in range(NPRE):
                raw = pre_pool.tile([128, 2 * D], mybir.dt.bfloat16,
                                    tag=f"raw{g}")
                nc.gpsimd.indirect_dma_start(
                    out=raw[:], out_offset=None, in_=seq.ap(),
                    in_offset=bass.IndirectOffsetOnAxis(
                        ap=pofs_sb[:, g:g + 1], axis=0))
                for j in range(2 * KT):
                    tp = tp_pool.tile([128, 128], mybir.dt.bfloat16, tag="tp")
                    nc.tensor.transpose(tp[:], raw[:, j * 128:(j + 1) * 128],
                                        id_sb[:])
                    nc.vector.tensor_copy(pre3[:, j, g * 128:(g + 1) * 128],
                                          tp[:])
            ps0 = psum_pool.tile([M, PRE], mybir.dt.float32, tag="ps")
            for t in range(NT):
                j = t if t < KT else t - KT
                nc.tensor.matmul(ps0[:], w3[:, t, :], pre3[:, j, :],
                                 start=(t == 0), stop=(t == NT - 1))
            o0 = out_pool.tile([M, PRE], mybir.dt.float32, tag="o")
            nc.vector.tensor_scalar_add(o0[:], ps0[:], b_sb[:])
            nc.sync.dma_start(out.ap()[:, 0:PRE], o0[:])

            # --- streamed tail: transpose-gathers over 4 SWDGE queues.
            off = PRE
            for c, sz in enumerate(SIZES):
                g_tile = gather_pool.tile([128, 2 * KT * sz], mybir.dt.bfloat16,
                                          tag="g")
                g3 = g_tile[:].rearrange("p (j n) -> p j n", j=2 * KT)
                nc.gpsimd.dma_gather(
                    g3,
                    seq.ap(),
                    idx_sb[:, off // 16:(off + sz) // 16],
                    sz,
                    sz,
                    2 * D,
                    transpose=True,
                    queue_num=QUEUES[c],
                )
                ps = psum_pool.tile([M, sz], mybir.dt.float32, tag="ps")
                for t in range(NT):
                    j = t if t < KT else t - KT
                    nc.tensor.matmul(ps[:], w3[:, t, :], g3[:, j, :],
                                     start=(t == 0), stop=(t == NT - 1))
                o_sb = out_pool.tile([M, sz], mybir.dt.float32, tag="o")
                nc.vector.tensor_scalar_add(o_sb[:], ps[:], b_sb[:])
                nc.sync.dma_start(out.ap()[:, off:off + sz], o_sb[:])
                off += sz
    nc.compile()
    return nc


def get_nc():
    if "nc" not in _cache:
        _cache["nc"] = _build()
    return _cache["nc"]


def host_prep(sequence_output, word_idxs, cls_w, cls_b):
    """Full inputs -> list of 8 per-core input maps."""
    seq = np.asarray(sequence_output, dtype=np.float32)          # [B, S, D]
    idxs = np.asarray(word_idxs).astype(np.int64)                # [B, W]
    w = np.asarray(cls_w, dtype=np.float32).reshape(M, D)        # row a*3+c
    b = np.asarray(cls_b, dtype=np.float32).reshape(M, 1)

    hi = seq.astype(ml_dtypes.bfloat16)
    lo = (seq - hi.astype(np.float32)).astype(ml_dtypes.bfloat16)
    seqcat = np.concatenate([hi, lo], axis=-1)                   # [B, S, 2D]

    whi = w.astype(ml_dtypes.bfloat16)
    wlo = (w - whi.astype(np.float32)).astype(ml_dtypes.bfloat16)
    whiT = whi.T.reshape(KT, 128, M).transpose(1, 0, 2)          # [p, kt, n]
    wloT = wlo.T.reshape(KT, 128, M).transpose(1, 0, 2)
    wcat = np.concatenate([whiT, wloT, whiT], axis=1)            # [128, NT, M]
    wcat = np.ascontiguousarray(wcat.reshape(128, NT * M))
    ident = np.eye(128, dtype=ml_dtypes.bfloat16)

    in_maps = []
    for c in range(NCORES):
        sl = slice(c * BLOC, (c + 1) * BLOC)
        seq_c = np.ascontiguousarray(seqcat[sl].reshape(BLOC * S, 2 * D))
        rows = (np.arange(BLOC)[:, None] * S + idxs[sl]).reshape(ROWS)
        # indirect pre-gather: group g, partition p <- row rows[g*128 + p]
        pofs = np.ascontiguousarray(
            rows[:PRE].reshape(NPRE, 128).T.astype(np.int32))
        # dma_gather: slot i reads its index from [i % 16, i // 16];
        # replicate the 16-partition pattern across all 128 partitions.
        idx_tile = np.tile(rows.reshape(ROWS // 16, 16).T.astype(np.int16),
                           (8, 1))
        in_maps.append({"seqcat": seq_c, "idx": np.ascontiguousarray(idx_tile),
                        "pofs": pofs, "wts": wcat, "bias": b, "ident": ident})
    return in_maps


def assemble(results):
    dev = np.stack([r["out"] for r in results])                  # [8, 96, ROWS]
    dev = dev.reshape(NCORES, A, 3, BLOC, W)
    return np.ascontiguousarray(
        dev.transpose(1, 0, 3, 4, 2).reshape(A, B, W, 3))


def run(inputs, trace=False):
    nc = get_nc()
    in_maps = host_prep(**inputs)
    res = bass_utils.run_bass_kernel_spmd(
        nc, in_maps, core_ids=list(range(NCORES)), trace=trace)
    return assemble(res.results), res


def kernel(sequence_output, word_idxs, cls_w, cls_b):
    out, _ = run(dict(sequence_output=sequence_output, word_idxs=word_idxs,
                      cls_w=cls_w, cls_b=cls_b))
    return out


# revision 19
# speedup vs baseline: 1.1762x; 1.1762x over previous
"""Trainium2 Bass kernel for BertForMultilabelNER head (gather + fused per-attribute classifier).

Reference computation:
    pooled = take_along_axis(sequence_output, word_idxs, axis=1)   # [B, W, D]
    logits = einsum("bwd,acd->abwc", pooled, cls_w) + cls_b        # [A, B, W, 3]

Strategy (8 NeuronCores, data-parallel over batch, 8 examples/core):
  * Host splits sequence_output rows into hi/lo bf16 planes (hi = bf16(x),
    lo = bf16(x - hi)) concatenated per row -> [B*S, 2D] bf16. Same bytes as
    f32, but enables dma_gather(transpose=True) (2-byte dtypes only), which
    gathers each indexed row directly into the [d%128 partitions, d//128, row]
    layout a matmul k-tile wants -- no on-chip transpose needed.
  * 18 accumulating bf16 matmuls per row-chunk: hi@w_hi + hi@w_lo + lo@w_hi
    (the dropped lo@w_lo term is ~2^-18 relative) ~= full f32 precision,
    at 1 cycle/row instead of f32's 4 cycles/row on the PE.
  * Gathers are split into chunks spread over the 4 SWDGE queues so the Q7
    pairs generate descriptors in parallel; the DVE adds the bias during the
    PSUM->SBUF copy; outputs stream back per chunk.
  * Device output is [96, 2048] per core; host reassembles [A, B, W, 3].

Measured on trn2 (8 cores): ~49us NEFF exec, of which ~17us is the fixed
GPSIMD library-overlay load that gates the first dma_gather and ~5us/4us are
the Tile preamble/exit barrier; the 6.3MB/core gather streams at ~300GB/s.
"""

import numpy as np
import ml_dtypes

import concourse.mybir as mybir
import concourse.tile as tile
from concourse import bacc, bass_utils

B, S, W, D, A = 64, 512, 256, 768, 32
NCORES = 8
BLOC = B // NCORES            # examples per core
ROWS = BLOC * W               # gathered rows per core (2048)
# Gather chunk sizes (rows). No gather can generate descriptors before the
# fixed ~17us GPSIMD library-load floor, so lead with big chunks (best
# transfer efficiency; generation for the first four runs in parallel on the
# 4 SWDGE queue pairs) and end small to shrink the matmul/writeback tail.
SIZES = [128, 128, 256, 256, 384, 384, 384, 128]
assert sum(SIZES) == ROWS
KT = D // 128                 # k-tiles per bf16 plane (6)
NT = 3 * KT                   # matmul k-tiles: hi*whi, hi*wlo, lo*whi (18)
M = 3 * A                     # output features (96)

_cache = {}


def _build():
    nc = bacc.Bacc("TRN2", target_bir_lowering=False, debug=False,
                   num_swdge_queues=4)
    seq = nc.dram_tensor("seqcat", [BLOC * S, 2 * D], mybir.dt.bfloat16,
                         kind="ExternalInput")
    idx = nc.dram_tensor("idx", [128, ROWS // 16], mybir.dt.int16,
                         kind="ExternalInput")
    wts = nc.dram_tensor("wts", [128, NT * M], mybir.dt.bfloat16,
                         kind="ExternalInput")
    bias = nc.dram_tensor("bias", [M, 1], mybir.dt.float32,
                          kind="ExternalInput")
    out = nc.dram_tensor("out", [M, ROWS], mybir.dt.float32,
                         kind="ExternalOutput")

    with tile.TileContext(nc) as tc:
        with (
            tc.tile_pool(name="const", bufs=1) as const_pool,
            tc.tile_pool(name="gather", bufs=len(SIZES)) as gather_pool,
            tc.tile_pool(name="psum", bufs=8, space="PSUM") as psum_pool,
            tc.tile_pool(name="outp", bufs=8) as out_pool,
        ):
            idx_sb = const_pool.tile([128, ROWS // 16], mybir.dt.int16)
            w_sb = const_pool.tile([128, NT * M], mybir.dt.bfloat16)
            b_sb = const_pool.tile([M, 1], mybir.dt.float32)
            nc.sync.dma_start(idx_sb[:], idx.ap())
            nc.sync.dma_start(w_sb[:], wts.ap())
            nc.sync.dma_start(b_sb[:], bias.ap())
            w3 = w_sb[:].rearrange("p (t n) -> p t n", t=NT)

            off = 0
            for c, sz in enumerate(SIZES):
                g_tile = gather_pool.tile([128, 2 * KT * sz], mybir.dt.bfloat16,
                                          tag="g")
                g3 = g_tile[:].rearrange("p (j n) -> p j n", j=2 * KT)
                nc.gpsimd.dma_gather(
                    g3,
                    seq.ap(),
                    idx_sb[:, off // 16:(off + sz) // 16],
                    sz,
                    sz,
                    2 * D,
                    transpose=True,
                    queue_num=c % 4,
                )
                ps = psum_pool.tile([M, sz], mybir.dt.float32, tag="ps")
                for t in range(NT):
                    j = t if t < KT else t - KT
                    nc.tensor.matmul(ps[:], w3[:, t, :], g3[:, j, :],
                                     start=(t == 0), stop=(t == NT - 1))
                o_sb = out_pool.tile([M, sz], mybir.dt.float32, tag="o")
                nc.vector.tensor_scalar_add(o_sb[:], ps[:], b_sb[:])
                nc.sync.dma_start(out.ap()[:, off:off + sz], o_sb[:])
                off += sz
    nc.compile()
    return nc


def get_nc():
    if "nc" not in _cache:
        _cache["nc"] = _build()
    return _cache["nc"]


def host_prep(sequence_output, word_idxs, cls_w, cls_b):
    """Full inputs -> list of 8 per-core input maps."""
    seq = np.asarray(sequence_output, dtype=np.float32)          # [B, S, D]
    idxs = np.asarray(word_idxs).astype(np.int64)                # [B, W]
    w = np.asarray(cls_w, dtype=np.float32).reshape(M, D)        # row a*3+c
    b = np.asarray(cls_b, dtype=np.float32).reshape(M, 1)

    hi = seq.astype(ml_dtypes.bfloat16)
    lo = (seq - hi.astype(np.float32)).astype(ml_dtypes.bfloat16)
    seqcat = np.concatenate([hi, lo], axis=-1)                   # [B, S, 2D]

    whi = w.astype(ml_dtypes.bfloat16)
    wlo = (w - whi.astype(np.float32)).astype(ml_dtypes.bfloat16)
    whiT = whi.T.reshape(KT, 128, M).transpose(1, 0, 2)          # [p, kt, n]
    wloT = wlo.T.reshape(KT, 128, M).transpose(1, 0, 2)
    wcat = np.concatenate([whiT, wloT, whiT], axis=1)            # [128, NT, M]
    wcat = np.ascontiguousarray(wcat.reshape(128, NT * M))

    in_maps = []
    for c in range(NCORES):
        sl = slice(c * BLOC, (c + 1) * BLOC)
        seq_c = np.ascontiguousarray(seqcat[sl].reshape(BLOC * S, 2 * D))
        rows = (np.arange(BLOC)[:, None] * S + idxs[sl]).reshape(ROWS)
        # gather slot i reads its index from [i % 16, i // 16]; replicate the
        # 16-partition pattern across all 128 partitions.
        idx_tile = np.tile(rows.reshape(ROWS // 16, 16).T.astype(np.int16),
                           (8, 1))
        in_maps.append({"seqcat": seq_c, "idx": np.ascontiguousarray(idx_tile),
                        "wts": wcat, "bias": b})
    return in_maps


def assemble(results):
    dev = np.stack([r["out"] for r in results])                  # [8, 96, ROWS]
    dev = dev.reshape(NCORES, A, 3, BLOC, W)
    return np.ascontiguousarray(
        dev.transpose(1, 0, 3, 4, 2).reshape(A, B, W, 3))


def run(inputs, trace=False):
    nc = get_nc()
    in_maps = host_prep(**inputs)
    res = bass_utils.run_bass_kernel_spmd(
        nc, in_maps, core_ids=list(range(NCORES)), trace=trace)
    return assemble(res.results), res


def kernel(sequence_output, word_idxs, cls_w, cls_b):
    out, _ = run(dict(sequence_output=sequence_output, word_idxs=word_idxs,
                      cls_w=cls_w, cls_b=cls_b))
    return out


# revision 20
# speedup vs baseline: 1.1764x; 1.0001x over previous
"""Trainium2 Bass kernel for BertForMultilabelNER head (gather + fused per-attribute classifier).

Reference computation:
    pooled = take_along_axis(sequence_output, word_idxs, axis=1)   # [B, W, D]
    logits = einsum("bwd,acd->abwc", pooled, cls_w) + cls_b        # [A, B, W, 3]

Strategy (8 NeuronCores, data-parallel over batch, 8 examples/core):
  * Host splits sequence_output rows into hi/lo bf16 planes (hi = bf16(x),
    lo = bf16(x - hi)) concatenated per row -> [B*S, 2D] bf16. Same bytes as
    f32, but enables dma_gather(transpose=True) (2-byte dtypes only), which
    gathers each indexed row directly into the [d%128 partitions, d//128, row]
    layout a matmul k-tile wants -- no on-chip transpose needed.
  * 18 accumulating bf16 matmuls per row-chunk: hi@w_hi + hi@w_lo + lo@w_hi
    (the dropped lo@w_lo term is ~2^-18 relative) ~= full f32 precision,
    at 1 cycle/row instead of f32's 4 cycles/row on the PE.
  * Gathers are split into chunks spread over the 4 SWDGE queues so the Q7
    pairs generate descriptors in parallel; the DVE adds the bias during the
    PSUM->SBUF copy; outputs stream back per chunk.
  * Device output is [96, 2048] per core; host reassembles [A, B, W, 3].

Measured on trn2 (8 cores): ~49us NEFF exec, of which ~17us is the fixed
GPSIMD library-overlay load that gates the first dma_gather and ~5us/4us are
the Tile preamble/exit barrier; the 6.3MB/core gather streams at ~300GB/s.
"""

import numpy as np
import ml_dtypes

import concourse.mybir as mybir
import concourse.tile as tile
from concourse import bacc, bass_utils, library_config

B, S, W, D, A = 64, 512, 256, 768, 32
NCORES = 8
BLOC = B // NCORES            # examples per core
ROWS = BLOC * W               # gathered rows per core (2048)
# Gather chunk sizes (rows). No gather can generate descriptors before the
# fixed ~17us GPSIMD library-load floor, so lead with big chunks (best
# transfer efficiency; generation for the first four runs in parallel on the
# 4 SWDGE queue pairs) and end small to shrink the matmul/writeback tail.
SIZES = [128, 128, 256, 256, 384, 384, 384, 128]
assert sum(SIZES) == ROWS
KT = D // 128                 # k-tiles per bf16 plane (6)
NT = 3 * KT                   # matmul k-tiles: hi*whi, hi*wlo, lo*whi (18)
M = 3 * A                     # output features (96)

_cache = {}


def _build():
    nc = bacc.Bacc("TRN2", target_bir_lowering=False, debug=False,
                   num_swdge_queues=4)
    seq = nc.dram_tensor("seqcat", [BLOC * S, 2 * D], mybir.dt.bfloat16,
                         kind="ExternalInput")
    idx = nc.dram_tensor("idx", [128, ROWS // 16], mybir.dt.int16,
                         kind="ExternalInput")
    wts = nc.dram_tensor("wts", [128, NT * M], mybir.dt.bfloat16,
                         kind="ExternalInput")
    bias = nc.dram_tensor("bias", [M, 1], mybir.dt.float32,
                          kind="ExternalInput")
    out = nc.dram_tensor("out", [M, ROWS], mybir.dt.float32,
                         kind="ExternalOutput")

    with tile.TileContext(nc) as tc:
        with (
            tc.tile_pool(name="const", bufs=1) as const_pool,
            tc.tile_pool(name="gather", bufs=len(SIZES)) as gather_pool,
            tc.tile_pool(name="psum", bufs=8, space="PSUM") as psum_pool,
            tc.tile_pool(name="outp", bufs=8) as out_pool,
        ):
            nc.gpsimd.load_library(library_config.mlp)
            idx_sb = const_pool.tile([128, ROWS // 16], mybir.dt.int16)
            w_sb = const_pool.tile([128, NT * M], mybir.dt.bfloat16)
            b_sb = const_pool.tile([M, 1], mybir.dt.float32)
            nc.sync.dma_start(idx_sb[:], idx.ap())
            nc.sync.dma_start(w_sb[:], wts.ap())
            nc.sync.dma_start(b_sb[:], bias.ap())
            w3 = w_sb[:].rearrange("p (t n) -> p t n", t=NT)

            off = 0
            for c, sz in enumerate(SIZES):
                g_tile = gather_pool.tile([128, 2 * KT * sz], mybir.dt.bfloat16,
                                          tag="g")
                g3 = g_tile[:].rearrange("p (j n) -> p j n", j=2 * KT)
                nc.gpsimd.dma_gather(
                    g3,
                    seq.ap(),
                    idx_sb[:, off // 16:(off + sz) // 16],
                    sz,
                    sz,
                    2 * D,
                    transpose=True,
                    queue_num=c % 4,
                )
                ps = psum_pool.tile([M, sz], mybir.dt.float32, tag="ps")
                for t in range(NT):
                    j = t if t < KT else t - KT
                    nc.tensor.matmul(ps[:], w3[:, t, :], g3[:, j, :],
                                     start=(t == 0), stop=(t == NT - 1))
                o_sb = out_pool.tile([M, sz], mybir.dt.float32, tag="o")
                nc.vector.tensor_scalar_add(o_sb[:], ps[:], b_sb[:])
                nc.sync.dma_start(out.ap()[:, off:off + sz], o_sb[:])
                off += sz
    nc.compile()
    return nc


def get_nc():
    if "nc" not in _cache:
        _cache["nc"] = _build()
    return _cache["nc"]


def host_prep(sequence_output, word_idxs, cls_w, cls_b):
    """Full inputs -> list of 8 per-core input maps."""
    seq = np.asarray(sequence_output, dtype=np.float32)          # [B, S, D]
    idxs = np.asarray(word_idxs).astype(np.int64)                # [B, W]
    w = np.asarray(cls_w, dtype=np.float32).reshape(M, D)        # row a*3+c
    b = np.asarray(cls_b, dtype=np.float32).reshape(M, 1)

    hi = seq.astype(ml_dtypes.bfloat16)
    lo = (seq - hi.astype(np.float32)).astype(ml_dtypes.bfloat16)
    seqcat = np.concatenate([hi, lo], axis=-1)                   # [B, S, 2D]

    whi = w.astype(ml_dtypes.bfloat16)
    wlo = (w - whi.astype(np.float32)).astype(ml_dtypes.bfloat16)
    whiT = whi.T.reshape(KT, 128, M).transpose(1, 0, 2)          # [p, kt, n]
    wloT = wlo.T.reshape(KT, 128, M).transpose(1, 0, 2)
    wcat = np.concatenate([whiT, wloT, whiT], axis=1)            # [128, NT, M]
    wcat = np.ascontiguousarray(wcat.reshape(128, NT * M))

    in_maps = []
    for c in range(NCORES):
        sl = slice(c * BLOC, (c + 1) * BLOC)
        seq_c = np.ascontiguousarray(seqcat[sl].reshape(BLOC * S, 2 * D))
        rows = (np.arange(BLOC)[:, None] * S + idxs[sl]).reshape(ROWS)
        # gather slot i reads its index from [i % 16, i // 16]; replicate the
        # 16-partition pattern across all 128 partitions.
        idx_tile = np.tile(rows.reshape(ROWS // 16, 16).T.astype(np.int16),
                           (8, 1))
        in_maps.append({"seqcat": seq_c, "idx": np.ascontiguousarray(idx_tile),
                        "wts": wcat, "bias": b})
    return in_maps


def assemble(results):
    dev = np.stack([r["out"] for r in results])                  # [8, 96, ROWS]
    dev = dev.reshape(NCORES, A, 3, BLOC, W)
    return np.ascontiguousarray(
        dev.transpose(1, 0, 3, 4, 2).reshape(A, B, W, 3))


def run(inputs, trace=False):
    nc = get_nc()
    in_maps = host_prep(**inputs)
    res = bass_utils.run_bass_kernel_spmd(
        nc, in_maps, core_ids=list(range(NCORES)), trace=trace)
    return assemble(res.results), res


def kernel(sequence_output, word_idxs, cls_w, cls_b):
    out, _ = run(dict(sequence_output=sequence_output, word_idxs=word_idxs,
                      cls_w=cls_w, cls_b=cls_b))
    return out
